# revision 3
# baseline (speedup 1.0000x reference)
"""TRN2 Bass kernel for nn_KnnModule (retrieval_knn).

Strategy (sharded over the 8 NeuronCores):
  - train set (100000 x 1024) is padded to 102400 rows and split into 8
    chunks of 12800; each core computes sims = features @ chunk.T
    (2048 x 12800) on the PE (float32r single-pass, 1 cycle/row), and for
    every 512-wide tile of the chunk extracts the top-8 values + indices
    per row with the DVE InstMax/InstMaxIndex top-8 primitives
    (25 tiles -> 200 candidates per row per core).
  - host merges the 8 x 200 per-row candidate lists, takes the top-48 by
    approximate value, recomputes their sims exactly in fp32 (48 dot
    products per row ~ 0.03% of the device FLOPs), and reproduces the
    reference softmax voting for k in (10, 20, 100, 200).

  Why this is exact: with T=0.07 and sims ~ N(0, 37^2), a candidate's
  fp32 softmax weight is exactly 0.0 unless its sim is within ~7.3 of the
  row max. On this distribution there are at most ~15 such candidates per
  row and at most 2 per 512-wide tile (4x safety margin vs the top-8
  extraction), and float32r's max matmul error (1.8e-2) is negligible vs
  the 7.3 window. Rows that get anywhere near these margins are detected
  and recomputed exactly on the host; on this data the triggers never
  fire.
"""

import numpy as np

KS = (10, 20, 100, 200)
T = 0.07
NUM_CLASSES = 1000
B, N, D = 2048, 100000, 1024
NCORES = 8
NCHUNK = 12800  # per-core padded chunk (12500 real + 300 zero pad)
TILE_N = 512
NT = NCHUNK // TILE_N  # 25 tiles -> 200 candidate slots per core
P = 128
KEXACT = 48  # candidates per row exactly rescored on host

_NC_CACHE = {}


def _build_bass():
    import concourse.bacc as bacc
    import concourse.mybir as mybir
    import concourse.tile as tile

    mm_dtype = mybir.dt.float32r
    KO = D // P
    MB = B // P

    nc = bacc.Bacc(
        "TRN2",
        target_bir_lowering=False,
        debug=False,
        enable_asserts=False,
    )
    featT = nc.dram_tensor("featT", (D, B), mm_dtype, kind="ExternalInput")
    trainT = nc.dram_tensor("trainT", (D, NCHUNK), mm_dtype, kind="ExternalInput")
    out_val = nc.dram_tensor("t8val", (B, NT * 8), mybir.dt.float32, kind="ExternalOutput")
    out_idx = nc.dram_tensor("t8idx", (B, NT * 8), mybir.dt.uint16, kind="ExternalOutput")

    featT_ap = featT.ap().rearrange("(ko p) b -> p ko b", p=P)
    trainT_ap = trainT.ap().rearrange("(ko p) n -> p ko n", p=P)

    with tile.TileContext(nc) as tc:
        with (
            tc.tile_pool(name="const", bufs=1) as cpool,
            tc.tile_pool(name="stream", bufs=3) as spool,
            tc.tile_pool(name="acc", bufs=1) as apool,
            tc.tile_pool(name="psum", bufs=8, space="PSUM") as ppool,
        ):
            feat_sb = cpool.tile([P, KO, B], mm_dtype)
            # chunked per m-block so the first matmuls don't wait on the
            # full 8MB features load (trace: 35us serial head otherwise)
            for m in range(MB):
                nc.sync.dma_start(
                    feat_sb[:, :, m * P : (m + 1) * P],
                    featT_ap[:, :, m * P : (m + 1) * P],
                )

            val_sb = [
                apool.tile([P, NT * 8], mybir.dt.float32, name=f"val_sb_{m}", tag=f"val{m}")
                for m in range(MB)
            ]
            idx_sb = [
                apool.tile([P, NT * 8], mybir.dt.uint16, name=f"idx_sb_{m}", tag=f"idx{m}")
                for m in range(MB)
            ]

            for t in range(NT):
                tr_sb = spool.tile([P, KO, TILE_N], mm_dtype, name="tr_sb", tag="train")
                nc.sync.dma_start(tr_sb, trainT_ap[:, :, t * TILE_N : (t + 1) * TILE_N])
                for m in range(MB):
                    ps = ppool.tile([P, TILE_N], mybir.dt.float32, name="ps", tag="ps")
                    for ko in range(KO):
                        nc.tensor.matmul(
                            ps,
                            lhsT=feat_sb[:, ko, m * P : (m + 1) * P],
                            rhs=tr_sb[:, ko],
                            start=(ko == 0),
                            stop=(ko == KO - 1),
                        )
                    vslice = val_sb[m][:, t * 8 : (t + 1) * 8]
                    nc.vector.max(out=vslice, in_=ps)
                    nc.vector.max_index(
                        out=idx_sb[m][:, t * 8 : (t + 1) * 8],
                        in_max=vslice,
                        in_values=ps,
                    )

            ov = out_val.ap().rearrange("(mb p) c -> mb p c", p=P)
            oi = out_idx.ap().rearrange("(mb p) c -> mb p c", p=P)
            for m in range(MB):
                nc.sync.dma_start(ov[m], val_sb[m])
                nc.sync.dma_start(oi[m], idx_sb[m])

    nc.compile()
    return nc


def _get_nc():
    if "nc" not in _NC_CACHE:
        _NC_CACHE["nc"] = _build_bass()
    return _NC_CACHE["nc"]


def _vote(topv, labels):
    """Reproduce the reference's softmax voting given sorted top sims.

    topv: (B', 200) fp32 descending (padded with -inf); labels (B', 200).
    """
    Bp = topv.shape[0]
    x = (topv / np.float32(T)).astype(np.float32)
    e = np.exp(x - x[:, :1], dtype=np.float32)
    s = e.sum(axis=1, keepdims=True, dtype=np.float32)
    w = (e / s).astype(np.float32)
    rows = np.broadcast_to(np.arange(Bp)[:, None], labels.shape)
    outs = []
    for k in KS:
        p = np.zeros((Bp, NUM_CLASSES), np.float32)
        np.add.at(p, (rows[:, :k], labels[:, :k]), w[:, :k])
        outs.append(p)
    return outs


def _exact_row(F, TR, LB, b):
    s = (F[b : b + 1] @ TR.T).astype(np.float32)[0]
    o = np.argsort(-s, kind="stable")[:200]
    return _vote(s[o][None].astype(np.float32), LB[o].astype(np.int64)[None])


def _combine(F, TR, LB, vals, idxs):
    NTN = NT * 8
    slot_tile = (np.arange(NTN) // 8) * TILE_N
    gcol = (
        idxs
        + slot_tile[None, None, :]
        + (np.arange(NCORES)[:, None, None] * NCHUNK)
    )
    v = vals.transpose(1, 0, 2).reshape(B, NCORES * NTN)
    g = gcol.transpose(1, 0, 2).reshape(B, NCORES * NTN)
    v = np.where(g < N, v, -np.inf).astype(np.float32)

    # approximate top-KEXACT per row
    part = np.argpartition(-v, KEXACT, axis=1)[:, :KEXACT]
    rows = np.arange(B)[:, None]
    cand_v = v[rows, part]
    cand_g = g[rows, part]

    # exact fp32 rescoring of the candidates (0.03% of device FLOPs)
    exact = np.einsum(
        "bkd,bd->bk", TR[cand_g], F, optimize=True
    ).astype(np.float32)

    # sort by exact value desc, ties by train index asc (lax.top_k order)
    ordk = np.lexsort((cand_g, -exact.astype(np.float64)), axis=1)
    exact_s = np.take_along_axis(exact, ordk, axis=1)
    g_s = np.take_along_axis(cand_g, ordk, axis=1)

    topv = np.full((B, 200), -np.inf, np.float32)
    topv[:, :KEXACT] = exact_s
    labels = np.zeros((B, 200), np.int64)
    labels[:, :KEXACT] = LB[g_s].astype(np.int64)

    outs = _vote(topv, labels)

    # pathological-row triggers -> exact host recompute
    amax = cand_v.max(axis=1)
    # (i) too many candidates near the top (exact-significance window overflow)
    near = (cand_v >= (amax[:, None] - 8.0)).sum(axis=1)
    trig_i = near >= KEXACT - 8
    # (ii) some tile's 8th approx value near the top (dropped 9th candidate)
    v8 = vals[:, :, 7::8]  # (ncores, B, NT)
    trig_ii = v8.max(axis=(0, 2)) >= amax - 8.5
    # (iii) duplicate global col among candidates (HW tie semantics)
    ss = np.sort(cand_g, axis=1)
    trig_iii = (np.diff(ss, axis=1) == 0).any(axis=1)

    for b in np.where(trig_i | trig_ii | trig_iii)[0]:
        ob = _exact_row(F, TR, LB, b)
        for i in range(len(KS)):
            outs[i][b] = ob[i][0]

    return tuple(outs)


def kernel(features_rank, train_features, train_labels):
    from concourse.bass_utils import run_bass_kernel_spmd

    F = np.ascontiguousarray(np.asarray(features_rank, dtype=np.float32))
    TR = np.ascontiguousarray(np.asarray(train_features, dtype=np.float32))
    LB = np.asarray(train_labels)

    TRp = np.zeros((NCORES * NCHUNK, D), np.float32)
    TRp[:N] = TR
    featT = np.ascontiguousarray(F.T)

    in_maps = [
        {
            "featT": featT,
            "trainT": np.ascontiguousarray(TRp[c * NCHUNK : (c + 1) * NCHUNK].T),
        }
        for c in range(NCORES)
    ]

    nc = _get_nc()
    res = run_bass_kernel_spmd(nc, in_maps, core_ids=list(range(NCORES)))

    vals = np.stack([np.asarray(res.results[c]["t8val"]) for c in range(NCORES)])
    idxs = np.stack(
        [np.asarray(res.results[c]["t8idx"]).astype(np.int64) for c in range(NCORES)]
    )
    return _combine(F, TR, LB, vals, idxs)
